# revision 20
# baseline (speedup 1.0000x reference)
"""Trainium2 Bass kernel for causal multi-head attention (GPT-style block).

Reference computation (fp32):
    qkv = x @ w_attn + b_attn          # [B,T,3C]
    q,k,v per head (12 heads, d=64)
    att = softmax(causal(q k^T / 8))
    y   = att @ v
    out = y @ w_proj + b_proj

Sharding: 8 cores = 2 batches x 4 head-groups (3 heads each).
Tensor-parallel over heads: each core takes its 3 heads' columns of
w_attn and rows of w_proj, computes a partial out^T [C, T]; the host
sums the 4 head-group partials per batch, transposes, adds b_proj.

Device kernel (per core), all matmuls bf16, PSUM fp32:

  1. x^T arrives pre-transposed [768, T]; 24 contiguous DMAs.
  2. Q/K^T = w^T x^T in 3 M-blocks: b0=[Q0|Q1] b1=[K0|K1] b2=[Q2|K2]
     ([128,T] SBUF tiles, | = partition 64). ACT-free bias add on DVE.
  3. V computed TOKEN-major directly: V[t,d] = x^T-tile.T @ wv
     (stationary = x^T block, moving = wv [128,192]) -> vk k-tiles
     [128, 3*65] (64 V cols + ones col per head; the ones column turns
     the AV matmul into a fused softmax-denominator sum).
  4. attention, software-pipelined one k-round ahead:
       round r=(qc,j):  S^T = K-slice.T @ Q   (PE, causal-sliced N)
                        + causal mask on diag strip (DVE)
                        eS = exp(0.125 S^T)     (ACT)
       and AV of round r-1: y^T[65,q] += [V|1].T @ eS  (PE)
     so no PE matmul ever waits on a just-issued exp.
  5. drain per q-chunk: 1/l via DVE reciprocal on the single l row,
     partition-broadcast on GpSimd (NOT a PE matmul), y^T *= rb (DVE).
  6. proj emits out^T: pso[128n,512t] = wp-slice.T @ y^T  (PE, N=512),
     DVE copy to SBUF, DMA out. Host transposes.
  Remaining qkv/V chains are woven between attention rounds as PE
  fillers so the tensor engine stays dense (p-state) while ACT exps
  pace the rounds.
"""

import os
import sys

import numpy as np

sys.path.insert(0, "/opt/trn_rl_repo")

from concourse import bacc, bass, mybir  # noqa: E402
from concourse import bass_utils  # noqa: E402
from concourse.tile import TileContext  # noqa: E402

P = 128
T = 2048
CDIM = 768
DHEAD = 64
HPC = 3  # heads per core
N_CORES = 8
FQK = 384  # q+k features per core (3 blocks of 128)
FV = 192  # v features per core
QW = 512  # q-chunk width
NQC = T // QW  # 4
NTT = T // P  # 16 token tiles
NCT = CDIM // P  # 6 contraction tiles
SCALE = 1.0 / np.sqrt(DHEAD)
NEG = -1.0e9

DT = mybir.dt.float32
DTM = mybir.dt.bfloat16


def build_nc():
    from contextlib import ExitStack

    nc = bacc.Bacc("TRN2", target_bir_lowering=False, debug=False)
    x_d = nc.dram_tensor("x", [CDIM, T], DTM, kind="ExternalInput")
    w_d = nc.dram_tensor("w", [CDIM, FQK + FV], DTM, kind="ExternalInput")
    bqk_d = nc.dram_tensor("bqk", [3, P], DT, kind="ExternalInput")
    bvb_d = nc.dram_tensor("bvb", [P, FV], DT, kind="ExternalInput")
    wp_d = nc.dram_tensor("wp", [HPC * DHEAD, CDIM], DTM, kind="ExternalInput")
    m_d = nc.dram_tensor("mask", [P, P], DT, kind="ExternalInput")
    onr_d = nc.dram_tensor("ones_r", [1, 64], mybir.dt.float32r, kind="ExternalInput")
    o_d = nc.dram_tensor("out", [CDIM, T], DT, kind="ExternalOutput")

    EXP = mybir.ActivationFunctionType.Exp

    with TileContext(nc) as tc, ExitStack() as ctx:
        const = ctx.enter_context(tc.tile_pool(name="const", bufs=1))
        big = ctx.enter_context(tc.tile_pool(name="big", bufs=1))
        work = ctx.enter_context(tc.tile_pool(name="work", bufs=4))
        psyp = ctx.enter_context(
            tc.tile_pool(name="psy", bufs=1, space=bass.MemorySpace.PSUM)
        )
        pssp = ctx.enter_context(
            tc.tile_pool(name="pss", bufs=1, space=bass.MemorySpace.PSUM)
        )
        misc = ctx.enter_context(
            tc.tile_pool(name="misc", bufs=2, space=bass.MemorySpace.PSUM)
        )

        # ---- input DMAs ----
        # x^T as [c-tile][t-chunk] tiles so consumers wait on single chunks
        xTt = [
            [
                big.tile([P, QW], DTM, tag=f"xT{c}_{t}", name=f"xT{c}_{t}")
                for t in range(NQC)
            ]
            for c in range(NCT)
        ]
        # w on the scalar ring, x t0 first on the sync ring, so the first
        # qkv chain pipelines with DMA arrival; remaining x chunks follow
        wt = []
        for c in range(NCT):
            w_c = const.tile([P, FQK + FV], DTM, tag=f"w{c}")
            nc.scalar.dma_start(w_c[:], w_d[c * P : (c + 1) * P, :])
            wt.append(w_c)
            nc.sync.dma_start(xTt[c][0][:], x_d[c * P : (c + 1) * P, 0:QW])
        bqk_t = const.tile([P, 3], DT, tag="bqk")
        nc.scalar.dma_start(bqk_t[:], bqk_d[:].rearrange("b p -> p b"))
        mask_t = const.tile([P, P], DT, tag="mask")
        nc.scalar.dma_start(mask_t[:], m_d[:])
        ones_r = const.tile([65, 64], mybir.dt.float32r, tag="ones_r")
        nc.scalar.dma_start(ones_r[64:65, :], onr_d[:])
        bvb_t = const.tile([P, FV], DT, tag="bvb")
        nc.scalar.dma_start(bvb_t[:], bvb_d[:])
        for t in range(1, NQC):
            for c in range(NCT):
                nc.sync.dma_start(
                    xTt[c][t][:],
                    x_d[c * P : (c + 1) * P, t * QW : (t + 1) * QW],
                )
        wp0 = const.tile([P, CDIM], DTM, tag="wp0")
        nc.scalar.dma_start(wp0[:], wp_d[0:P, :])
        wp1 = const.tile([64, CDIM], DTM, tag="wp1")
        nc.scalar.dma_start(wp1[:], wp_d[P : P + 64, :])

        # ---- V k-tiles; ones column per head fused for the l-sum ----
        vk = [
            big.tile([P, HPC * 65], DTM, tag=f"vk{j}", name=f"vk{j}")
            for j in range(NTT)
        ]
        for j in range(NTT):
            ones_view = vk[j][:].rearrange("p (h c) -> p h c", c=65)[:, :, 64:65]
            nc.gpsimd.memset(ones_view, 1.0)

        # ---- chain emitters (each is a PE filler unit) ----
        blk = [big.tile([P, T], DTM, tag=f"blk{i}", name=f"blk{i}") for i in range(3)]
        # K2 copy at partition base 0 (matmul needs lhsT/rhs bases equal;
        # Q2 lives at base 0 in blk2, K2 at base 64 -> shift via DMA)
        kT2 = big.tile([64, T], DTM, tag="kT2", name="kT2")

        # Chains are emitted in 2-matmul units so they can be spread as PE
        # gap fillers between attention rounds (ACT exp stream paces the
        # rounds; PE must stay dense for p-state). A unit emits ~2 matmuls
        # and returns its estimated PE-ns.
        qkv_state = {}

        def qkv_unit(t, bi, half):
            tcols = slice(t * QW, (t + 1) * QW)
            if half == 0:
                qkv_state[(t, bi)] = misc.tile([P, QW], DT, tag="misc", name="psq")
            psq = qkv_state[(t, bi)]
            for c in (half * 2, half * 2 + 1) if half < 2 else (4, 5):
                nc.tensor.matmul(
                    psq[:],
                    wt[c][:, bi * P : (bi + 1) * P],
                    xTt[c][t][:],
                    start=(c == 0),
                    stop=(c == NCT - 1),
                )
            if half == 2:
                nc.vector.tensor_scalar_add(
                    blk[bi][:, tcols], psq[:], bqk_t[:, bi : bi + 1]
                )
                if bi == 2:
                    nc.sync.dma_start(kT2[:, tcols], blk[2][64:P, tcols])
            return 540

        v_state = {}

        def v_unit(j, half):
            tq, to = divmod(j, 4)
            if half == 0:
                v_state[j] = misc.tile([P, FV], DT, tag="misc", name="pst")
            pst = v_state[j]
            for c in (half * 2, half * 2 + 1) if half < 2 else (4, 5):
                nc.tensor.matmul(
                    pst[:],
                    xTt[c][tq][:, to * P : (to + 1) * P],
                    wt[c][:, FQK : FQK + FV],
                    start=(c == 0),
                    stop=(c == NCT - 1),
                )
            if half == 2:
                data_view = vk[j][:].rearrange("p (h c) -> p h c", c=65)[:, :, 0:64]
                nc.vector.tensor_add(
                    data_view,
                    pst[:].rearrange("p (h c) -> p h c", c=64),
                    bvb_t[:].rearrange("p (h c) -> p h c", c=64),
                )
            return 270

        # ---- attention machinery ----
        # Q head slices (block idx, row base); K source APs per head
        qsrc = [(0, 0), (0, 64), (2, 0)]
        ktile = [blk[1], blk[1], kT2]
        krow = [0, 64, 0]
        yT0 = big.tile([P, T], DTM, tag="yT0")  # rows: h0 | h1
        yT1 = big.tile([64, T], DTM, tag="yT1")  # h2
        psy = [None, None, None]

        def emit_s_round(qc, j):
            m = j - 4 * qc
            cs = m * P if m >= 1 else 0
            ssl = slice(cs, QW)
            qsl = slice(qc * QW + cs, (qc + 1) * QW)
            es3 = []
            for h in range(HPC):
                qb, qr = qsrc[h]
                kr = krow[h]
                pss = pssp.tile([P, QW], DT, tag=f"pss{h}", name=f"pss{h}")
                nc.tensor.matmul(
                    pss[:, ssl],
                    ktile[h][kr : kr + 64, j * P : (j + 1) * P],
                    blk[qb][qr : qr + 64, qsl],
                    start=True,
                    stop=True,
                )
                if m >= 0:
                    msl = slice(m * P, (m + 1) * P)
                    nc.vector.tensor_add(pss[:, msl], pss[:, msl], mask_t[:])
                es = work.tile([P, QW], DTM, tag="es", bufs=9, name="es")
                nc.scalar.activation(es[:, ssl], pss[:, ssl], EXP, scale=float(SCALE))
                es3.append(es)
            return (qc, j, es3, ssl)

        def emit_av_round(qc, j, es3, ssl):
            first = j == 0
            last = j == 4 * (qc + 1) - 1
            if first:
                for h in range(HPC):
                    psy[h] = psyp.tile([65, QW], DT, tag=f"psy{h}", name=f"psy{h}")
            for h in range(HPC):
                nc.tensor.matmul(
                    psy[h][:, ssl],
                    vk[j][:, 65 * h : 65 * h + 65],
                    es3[h][:, ssl],
                    start=first,
                    stop=last,
                )
            return last

        def drain_unit(qc, h):
            # custom-DVE ops (reciprocal) are broken at partition base != 0
            # on HW, so: copy l (f32r), PE K=1 matmul broadcast to
            # partitions 0:64, reciprocal at base 0, then normalize.
            qcols = slice(qc * QW, (qc + 1) * QW)
            lr = work.tile([65, QW], mybir.dt.float32r, tag="lr", bufs=2)
            nc.vector.tensor_copy(lr[64:65, :], psy[h][64:65, :])
            rbp = misc.tile([64, QW], DT, tag="misc", name="rbp")
            nc.tensor.matmul(
                rbp[:], ones_r[64:65, :], lr[64:65, :], start=True, stop=True
            )
            rb = work.tile([64, QW], DT, tag="rb", bufs=2)
            nc.vector.reciprocal_approx_fast(out=rb[:], in_=rbp[:])
            if h == 0:
                nc.vector.tensor_mul(yT0[0:64, qcols], psy[h][0:64, :], rb[:])
            elif h == 2:
                nc.vector.tensor_mul(yT1[0:64, qcols], psy[h][0:64, :], rb[:])
            else:
                tmp = work.tile([64, QW], DTM, tag="ytmp", bufs=2)
                nc.vector.tensor_mul(tmp[:], psy[h][0:64, :], rb[:])
                nc.sync.dma_start(yT0[64:P, qcols], tmp[:])
            return 300

        def proj_unit(qc, ns):
            tsl = slice(qc * QW, (qc + 1) * QW)
            nsl = slice(ns * P, (ns + 1) * P)
            pso = misc.tile([P, QW], DT, tag="misc")
            nc.tensor.matmul(pso[:], wp0[:, nsl], yT0[:, tsl], start=True, stop=False)
            nc.tensor.matmul(pso[:], wp1[:, nsl], yT1[:, tsl], start=False, stop=True)
            ot = work.tile([P, QW], DT, tag="ot", bufs=3)
            nc.vector.tensor_copy(ot[:], pso[:])
            nc.sync.dma_start(o_d[nsl, tsl], ot[:])
            return 520

        # ---- prologue: only what round (0,0) needs ----
        for bi in range(3):
            for half in range(3):
                qkv_unit(0, bi, half)

        def qkv_chain(t, bi):
            for half in range(3):
                qkv_unit(t, bi, half)
            return 1620

        def v_chain(j):
            for half in range(3):
                v_unit(j, half)
            return 810

        # ---- filler queue with deadlines ----
        # each entry: (deadline_round_index, thunk). Chains pop atomically;
        # pacing is even distribution (PE total work > ACT total, so the
        # queue must drain by the last round, not just plug ACT gaps).
        rounds = [(qc, j) for qc in range(NQC) for j in range(4 * (qc + 1))]
        ridx = {r: i for i, r in enumerate(rounds)}
        NR = len(rounds)
        queue = []
        for j in range(4):
            # vk[j] first consumed by AV of round (0, j), emitted one
            # round later
            queue.append((ridx[(0, j)], lambda j=j: v_chain(j)))
        for t in range(1, 4):
            for bi in range(3):
                # Q/K for chunk t read by round (t, 0)'s S, which is emitted
                # at the START of that round body -> pop one round earlier
                queue.append((ridx[(t, 0)] - 1, lambda t=t, bi=bi: qkv_chain(t, bi)))
        for j in range(4, NTT):
            queue.append((ridx[(j // 4, j)], lambda j=j: v_chain(j)))
        queue.sort(key=lambda e: e[0])

        # ---- main loop: S(r) + AV(r-2) + evenly-paced fillers ----
        pends = []
        for ri, (qc, j) in enumerate(rounds):
            cur = emit_s_round(qc, j)
            if len(pends) >= 2:
                pend = pends.pop(0)
                was_last = emit_av_round(*pend)
                if was_last:
                    pqc = pend[0]
                    for h in (1, 0, 2):  # h1 first: longest chain (DMA shift)
                        drain_unit(pqc, h)
                    for ns in range(6):
                        queue.append((NR - 1, lambda q=pqc, n=ns: proj_unit(q, n)))
            # even pacing: drain the queue by the end; deadlines force early
            npop = max(0, (len(queue) + (NR - 1 - ri)) // max(1, NR - ri))
            while queue and (queue[0][0] <= ri or npop > 0):
                _, thunk = queue.pop(0)
                thunk()
                npop -= 1
            pends.append(cur)
        for pend in pends:
            was_last = emit_av_round(*pend)
            if was_last and pend[0] < NQC - 1:
                for h in (1, 0, 2):
                    drain_unit(pend[0], h)
                for ns in range(6):
                    queue.append((NR - 1, lambda q=pend[0], n=ns: proj_unit(q, n)))
        for h in (1, 0, 2):
            drain_unit(NQC - 1, h)
        for _, thunk in queue:
            thunk()
        for ns in range(6):
            proj_unit(NQC - 1, ns)

    nc.compile()
    return nc


_NC_CACHE = None


def _get_nc():
    global _NC_CACHE
    if _NC_CACHE is None:
        _NC_CACHE = build_nc()
    return _NC_CACHE


def _host_inputs(x, w_attn, b_attn, w_proj):
    """Per-core input dicts. Core c = batch (c//4), head-group (c%4)."""
    import ml_dtypes

    npm = ml_dtypes.bfloat16
    x = np.ascontiguousarray(np.asarray(x, dtype=np.float32))
    w_attn = np.asarray(w_attn, dtype=np.float32)
    b_attn = np.asarray(b_attn, dtype=np.float32)
    w_proj = np.asarray(w_proj, dtype=np.float32)

    # triangular causal mask tile [128, 128]: p > f -> NEG
    pp, ff = np.meshgrid(np.arange(P), np.arange(P), indexing="ij")
    mask = np.where(pp > ff, np.float32(NEG), np.float32(0.0)).astype(np.float32)

    in_maps = []
    for core in range(N_CORES):
        b, hg = divmod(core, 4)
        hs = 3 * hg  # first head of this core
        # column bases in the 2304-wide qkv dim
        q0, k0, v0 = 64 * hs, CDIM + 64 * hs, 2 * CDIM + 64 * hs
        # M-blocks: b0=[Q0|Q1] b1=[K0|K1] b2=[Q2|K2], then V (192)
        w = np.concatenate(
            [
                w_attn[:, q0 : q0 + 128],
                w_attn[:, k0 : k0 + 128],
                w_attn[:, q0 + 128 : q0 + 192],
                w_attn[:, k0 + 128 : k0 + 192],
                w_attn[:, v0 : v0 + 192],
            ],
            axis=1,
        )
        bqk = np.zeros((3, P), dtype=np.float32)
        bqk[0] = b_attn[q0 : q0 + 128]
        bqk[1] = b_attn[k0 : k0 + 128]
        bqk[2, 0:64] = b_attn[q0 + 128 : q0 + 192]
        bqk[2, 64:128] = b_attn[k0 + 128 : k0 + 192]
        bvb = np.tile(b_attn[v0 : v0 + 192][None, :], (P, 1)).astype(np.float32)
        wp = np.ascontiguousarray(w_proj[64 * hs : 64 * hs + 192, :])
        in_maps.append(
            {
                "x": np.ascontiguousarray(x[b].T.astype(npm)),
                "w": np.ascontiguousarray(w.astype(npm)),
                "bqk": bqk,
                "bvb": bvb,
                "wp": wp.astype(npm),
                "mask": mask,
                "ones_r": np.ones((1, 64), dtype=np.float32),
            }
        )
    return in_maps


def run(x, w_attn, b_attn, w_proj, b_proj, trace=False):
    nc = _get_nc()
    in_maps = _host_inputs(x, w_attn, b_attn, w_proj)
    res = bass_utils.run_bass_kernel_spmd(
        nc, in_maps, core_ids=list(range(N_CORES)), trace=trace
    )
    B = 2
    outT = np.zeros((B, CDIM, T), dtype=np.float64)
    for core in range(N_CORES):
        outT[core // 4] += res.results[core]["out"].astype(np.float64)
    out = outT.transpose(0, 2, 1)
    out = out + np.asarray(b_proj, dtype=np.float64)[None, None, :]
    return np.ascontiguousarray(out.astype(np.float32)), res


def kernel(x, w_attn, b_attn, w_proj, b_proj):
    out, _ = run(x, w_attn, b_attn, w_proj, b_proj, trace=False)
    return out


# revision 21
# speedup vs baseline: 1.2082x; 1.2082x over previous
"""Trainium2 Bass kernel for causal multi-head attention (GPT-style block).

Reference computation (fp32):
    qkv = x @ w_attn + b_attn          # [B,T,3C]
    q,k,v per head (12 heads, d=64)
    att = softmax(causal(q k^T / 8))
    y   = att @ v
    out = y @ w_proj + b_proj

Sharding: 8 cores = 2 batches x 4 head-groups (3 heads each).
Tensor-parallel over heads: each core takes its 3 heads' columns of
w_attn and rows of w_proj, computes a partial out^T [C, T]; the host
sums the 4 head-group partials per batch, transposes, adds b_proj.

Device kernel (per core), all matmuls bf16, PSUM fp32:

  1. x^T arrives pre-transposed [768, T]; 24 contiguous DMAs.
  2. Q/K^T = w^T x^T in 3 M-blocks: b0=[Q0|Q1] b1=[K0|K1] b2=[Q2|K2]
     ([128,T] SBUF tiles, | = partition 64). ACT-free bias add on DVE.
  3. V computed TOKEN-major directly: V[t,d] = x^T-tile.T @ wv
     (stationary = x^T block, moving = wv [128,192]) -> vk k-tiles
     [128, 3*65] (64 V cols + ones col per head; the ones column turns
     the AV matmul into a fused softmax-denominator sum).
  4. attention, software-pipelined one k-round ahead:
       round r=(qc,j):  S^T = K-slice.T @ Q   (PE, causal-sliced N)
                        + causal mask on diag strip (DVE)
                        eS = exp(0.125 S^T)     (ACT)
       and AV of round r-1: y^T[65,q] += [V|1].T @ eS  (PE)
     so no PE matmul ever waits on a just-issued exp.
  5. drain per q-chunk: 1/l via DVE reciprocal on the single l row,
     partition-broadcast on GpSimd (NOT a PE matmul), y^T *= rb (DVE).
  6. proj emits out^T: pso[128n,512t] = wp-slice.T @ y^T  (PE, N=512),
     DVE copy to SBUF, DMA out. Host transposes.
  Remaining qkv/V chains are woven between attention rounds as PE
  fillers so the tensor engine stays dense (p-state) while ACT exps
  pace the rounds.
"""

import os
import sys

import numpy as np

sys.path.insert(0, "/opt/trn_rl_repo")

from concourse import bacc, bass, mybir  # noqa: E402
from concourse import bass_utils  # noqa: E402
from concourse.tile import TileContext  # noqa: E402

P = 128
T = 2048
CDIM = 768
DHEAD = 64
HPC = 3  # heads per core
N_CORES = 8
FQK = 384  # q+k features per core (3 blocks of 128)
FV = 192  # v features per core
QW = 512  # q-chunk width
NQC = T // QW  # 4
NTT = T // P  # 16 token tiles
NCT = CDIM // P  # 6 contraction tiles
SCALE = 1.0 / np.sqrt(DHEAD)
NEG = -1.0e9

DT = mybir.dt.float32
DTM = mybir.dt.bfloat16


def build_nc():
    from contextlib import ExitStack

    nc = bacc.Bacc("TRN2", target_bir_lowering=False, debug=False)
    x_d = nc.dram_tensor("x", [CDIM, T], DTM, kind="ExternalInput")
    w_d = nc.dram_tensor("w", [CDIM, FQK + FV], DTM, kind="ExternalInput")
    bqk_d = nc.dram_tensor("bqk", [3, P], DT, kind="ExternalInput")
    bvb_d = nc.dram_tensor("bvb", [P, FV], DT, kind="ExternalInput")
    wp_d = nc.dram_tensor("wp", [HPC * DHEAD, CDIM], DTM, kind="ExternalInput")
    m_d = nc.dram_tensor("mask", [P, P], DT, kind="ExternalInput")
    onr_d = nc.dram_tensor("ones_r", [1, 64], mybir.dt.float32r, kind="ExternalInput")
    o_d = nc.dram_tensor("out", [CDIM, T], DT, kind="ExternalOutput")

    EXP = mybir.ActivationFunctionType.Exp

    with TileContext(nc) as tc, ExitStack() as ctx:
        const = ctx.enter_context(tc.tile_pool(name="const", bufs=1))
        big = ctx.enter_context(tc.tile_pool(name="big", bufs=1))
        work = ctx.enter_context(tc.tile_pool(name="work", bufs=4))
        psyp = ctx.enter_context(
            tc.tile_pool(name="psy", bufs=1, space=bass.MemorySpace.PSUM)
        )
        pssp = ctx.enter_context(
            tc.tile_pool(name="pss", bufs=1, space=bass.MemorySpace.PSUM)
        )
        misc = ctx.enter_context(
            tc.tile_pool(name="misc", bufs=2, space=bass.MemorySpace.PSUM)
        )

        # ---- input DMAs ----
        # x^T as [c-tile][t-chunk] tiles so consumers wait on single chunks
        xTt = [
            [
                big.tile([P, QW], DTM, tag=f"xT{c}_{t}", name=f"xT{c}_{t}")
                for t in range(NQC)
            ]
            for c in range(NCT)
        ]
        # w on the scalar ring, x t0 first on the sync ring, so the first
        # qkv chain pipelines with DMA arrival; remaining x chunks follow
        wt = []
        for c in range(NCT):
            w_c = const.tile([P, FQK + FV], DTM, tag=f"w{c}")
            nc.scalar.dma_start(w_c[:], w_d[c * P : (c + 1) * P, :])
            wt.append(w_c)
            nc.sync.dma_start(xTt[c][0][:], x_d[c * P : (c + 1) * P, 0:QW])
        bqk_t = const.tile([P, 3], DT, tag="bqk")
        nc.scalar.dma_start(bqk_t[:], bqk_d[:].rearrange("b p -> p b"))
        mask_t = const.tile([P, P], DT, tag="mask")
        nc.scalar.dma_start(mask_t[:], m_d[:])
        ones_r = const.tile([65, 64], mybir.dt.float32r, tag="ones_r")
        nc.scalar.dma_start(ones_r[64:65, :], onr_d[:])
        bvb_t = const.tile([P, FV], DT, tag="bvb")
        nc.scalar.dma_start(bvb_t[:], bvb_d[:])
        for t in range(1, NQC):
            for c in range(NCT):
                nc.sync.dma_start(
                    xTt[c][t][:],
                    x_d[c * P : (c + 1) * P, t * QW : (t + 1) * QW],
                )
        wp0 = const.tile([P, CDIM], DTM, tag="wp0")
        nc.scalar.dma_start(wp0[:], wp_d[0:P, :])
        wp1 = const.tile([64, CDIM], DTM, tag="wp1")
        nc.scalar.dma_start(wp1[:], wp_d[P : P + 64, :])

        # ---- V k-tiles; ones column per head fused for the l-sum ----
        vk = [
            big.tile([P, HPC * 65], DTM, tag=f"vk{j}", name=f"vk{j}")
            for j in range(NTT)
        ]
        for j in range(NTT):
            ones_view = vk[j][:].rearrange("p (h c) -> p h c", c=65)[:, :, 64:65]
            nc.gpsimd.memset(ones_view, 1.0)

        # ---- chain emitters (each is a PE filler unit) ----
        blk = [big.tile([P, T], DTM, tag=f"blk{i}", name=f"blk{i}") for i in range(3)]
        # K2 copy at partition base 0 (matmul needs lhsT/rhs bases equal;
        # Q2 lives at base 0 in blk2, K2 at base 64 -> shift via DMA)
        kT2 = big.tile([64, T], DTM, tag="kT2", name="kT2")

        # Chains are emitted in 2-matmul units so they can be spread as PE
        # gap fillers between attention rounds (ACT exp stream paces the
        # rounds; PE must stay dense for p-state). A unit emits ~2 matmuls
        # and returns its estimated PE-ns.
        qkv_state = {}

        def qkv_unit(t, bi, half):
            tcols = slice(t * QW, (t + 1) * QW)
            if half == 0:
                qkv_state[(t, bi)] = misc.tile([P, QW], DT, tag="misc", name="psq")
            psq = qkv_state[(t, bi)]
            for c in (half * 2, half * 2 + 1) if half < 2 else (4, 5):
                nc.tensor.matmul(
                    psq[:],
                    wt[c][:, bi * P : (bi + 1) * P],
                    xTt[c][t][:],
                    start=(c == 0),
                    stop=(c == NCT - 1),
                )
            if half == 2:
                nc.vector.tensor_scalar_add(
                    blk[bi][:, tcols], psq[:], bqk_t[:, bi : bi + 1]
                )
                if bi == 2:
                    nc.sync.dma_start(kT2[:, tcols], blk[2][64:P, tcols])
            return 540

        v_state = {}

        def v_unit(j, half):
            tq, to = divmod(j, 4)
            if half == 0:
                v_state[j] = misc.tile([P, FV], DT, tag="misc", name="pst")
            pst = v_state[j]
            for c in (half * 2, half * 2 + 1) if half < 2 else (4, 5):
                nc.tensor.matmul(
                    pst[:],
                    xTt[c][tq][:, to * P : (to + 1) * P],
                    wt[c][:, FQK : FQK + FV],
                    start=(c == 0),
                    stop=(c == NCT - 1),
                )
            if half == 2:
                data_view = vk[j][:].rearrange("p (h c) -> p h c", c=65)[:, :, 0:64]
                nc.vector.tensor_add(
                    data_view,
                    pst[:].rearrange("p (h c) -> p h c", c=64),
                    bvb_t[:].rearrange("p (h c) -> p h c", c=64),
                )
            return 270

        # ---- attention machinery ----
        # Q head slices (block idx, row base); K source APs per head
        qsrc = [(0, 0), (0, 64), (2, 0)]
        ktile = [blk[1], blk[1], kT2]
        krow = [0, 64, 0]
        yT0 = big.tile([P, T], DTM, tag="yT0")  # rows: h0 | h1
        yT1 = big.tile([64, T], DTM, tag="yT1")  # h2
        psy = [None, None, None]

        def emit_s_round(qc, j):
            m = j - 4 * qc
            cs = m * P if m >= 1 else 0
            ssl = slice(cs, QW)
            qsl = slice(qc * QW + cs, (qc + 1) * QW)
            es3 = []
            for h in range(HPC):
                qb, qr = qsrc[h]
                kr = krow[h]
                pss = pssp.tile([P, QW], DT, tag=f"pss{h}", name=f"pss{h}")
                nc.tensor.matmul(
                    pss[:, ssl],
                    ktile[h][kr : kr + 64, j * P : (j + 1) * P],
                    blk[qb][qr : qr + 64, qsl],
                    start=True,
                    stop=True,
                )
                if m >= 0:
                    msl = slice(m * P, (m + 1) * P)
                    nc.vector.tensor_add(pss[:, msl], pss[:, msl], mask_t[:])
                es = work.tile([P, QW], DTM, tag="es", bufs=9, name="es")
                nc.scalar.activation(es[:, ssl], pss[:, ssl], EXP, scale=float(SCALE))
                es3.append(es)
            return (qc, j, es3, ssl)

        def emit_av_round(qc, j, es3, ssl):
            first = j == 0
            last = j == 4 * (qc + 1) - 1
            if first:
                for h in range(HPC):
                    psy[h] = psyp.tile([65, QW], DT, tag=f"psy{h}", name=f"psy{h}")
            for h in range(HPC):
                nc.tensor.matmul(
                    psy[h][:, ssl],
                    vk[j][:, 65 * h : 65 * h + 65],
                    es3[h][:, ssl],
                    start=first,
                    stop=last,
                )
            return last

        def drain_unit(qc, h):
            # Free psy FAST: two DVE copies move l + unnormalized y out of
            # PSUM so the next q-chunk's AV can reuse the bank immediately.
            # The 1/l chain (PE K=1 broadcast -> reciprocal at base 0 ->
            # in-place normalize) then runs off the critical path; only
            # proj(qc) waits on it. (custom-DVE reciprocal is broken at
            # partition base != 0 on HW, hence the PE broadcast.)
            qcols = slice(qc * QW, (qc + 1) * QW)
            lr = work.tile([65, QW], mybir.dt.float32r, tag="lr", bufs=2)
            nc.vector.tensor_copy(lr[64:65, :], psy[h][64:65, :])
            if h == 0:
                ydst = yT0[0:64, qcols]
            elif h == 2:
                ydst = yT1[0:64, qcols]
            else:
                tmp = work.tile([64, QW], DTM, tag="ytmp", bufs=2)
                ydst = tmp[:]
            nc.vector.tensor_copy(ydst, psy[h][0:64, :])
            rbp = misc.tile([64, QW], DT, tag="misc", name="rbp")
            nc.tensor.matmul(
                rbp[:], ones_r[64:65, :], lr[64:65, :], start=True, stop=True
            )
            rb = work.tile([64, QW], DT, tag="rb", bufs=2)
            nc.vector.reciprocal_approx_fast(out=rb[:], in_=rbp[:])
            nc.vector.tensor_mul(ydst, ydst, rb[:])
            if h == 1:
                nc.sync.dma_start(yT0[64:P, qcols], tmp[:])
            return 300

        def proj_unit(qc, ns):
            tsl = slice(qc * QW, (qc + 1) * QW)
            nsl = slice(ns * P, (ns + 1) * P)
            pso = misc.tile([P, QW], DT, tag="misc")
            nc.tensor.matmul(pso[:], wp0[:, nsl], yT0[:, tsl], start=True, stop=False)
            nc.tensor.matmul(pso[:], wp1[:, nsl], yT1[:, tsl], start=False, stop=True)
            ot = work.tile([P, QW], DT, tag="ot", bufs=3)
            nc.vector.tensor_copy(ot[:], pso[:])
            nc.sync.dma_start(o_d[nsl, tsl], ot[:])
            return 520

        # ---- prologue: only what round (0,0) needs ----
        for bi in range(3):
            for half in range(3):
                qkv_unit(0, bi, half)

        def qkv_chain(t, bi):
            for half in range(3):
                qkv_unit(t, bi, half)
            return 1620

        def v_chain(j):
            for half in range(3):
                v_unit(j, half)
            return 810

        # ---- filler queue with deadlines ----
        # each entry: (deadline_round_index, thunk). Chains pop atomically;
        # pacing is even distribution (PE total work > ACT total, so the
        # queue must drain by the last round, not just plug ACT gaps).
        rounds = [(qc, j) for qc in range(NQC) for j in range(4 * (qc + 1))]
        ridx = {r: i for i, r in enumerate(rounds)}
        NR = len(rounds)
        queue = []
        for j in range(4):
            # vk[j] first consumed by AV of round (0, j), emitted one
            # round later
            queue.append((ridx[(0, j)], lambda j=j: v_chain(j)))
        for t in range(1, 4):
            for bi in range(3):
                # Q/K for chunk t read by round (t, 0)'s S, which is emitted
                # at the START of that round body -> pop one round earlier
                queue.append((ridx[(t, 0)] - 1, lambda t=t, bi=bi: qkv_chain(t, bi)))
        for j in range(4, NTT):
            queue.append((ridx[(j // 4, j)], lambda j=j: v_chain(j)))
        queue.sort(key=lambda e: e[0])

        # ---- main loop: S(r) + AV(r-2) + evenly-paced fillers ----
        pends = []
        for ri, (qc, j) in enumerate(rounds):
            cur = emit_s_round(qc, j)
            if len(pends) >= 2:
                pend = pends.pop(0)
                was_last = emit_av_round(*pend)
                if was_last:
                    pqc = pend[0]
                    for h in (1, 0, 2):  # h1 first: longest chain (DMA shift)
                        drain_unit(pqc, h)
                    for ns in range(6):
                        queue.append((NR - 1, lambda q=pqc, n=ns: proj_unit(q, n)))
            # even pacing: drain the queue by the end; deadlines force early
            npop = max(0, (len(queue) + (NR - 1 - ri)) // max(1, NR - ri))
            while queue and (queue[0][0] <= ri or npop > 0):
                _, thunk = queue.pop(0)
                thunk()
                npop -= 1
            pends.append(cur)
        for pend in pends:
            was_last = emit_av_round(*pend)
            if was_last and pend[0] < NQC - 1:
                for h in (1, 0, 2):
                    drain_unit(pend[0], h)
                for ns in range(6):
                    queue.append((NR - 1, lambda q=pend[0], n=ns: proj_unit(q, n)))
        for h in (1, 0, 2):
            drain_unit(NQC - 1, h)
        for _, thunk in queue:
            thunk()
        for ns in range(6):
            proj_unit(NQC - 1, ns)

    nc.compile()
    return nc


_NC_CACHE = None


def _get_nc():
    global _NC_CACHE
    if _NC_CACHE is None:
        _NC_CACHE = build_nc()
    return _NC_CACHE


def _host_inputs(x, w_attn, b_attn, w_proj):
    """Per-core input dicts. Core c = batch (c//4), head-group (c%4)."""
    import ml_dtypes

    npm = ml_dtypes.bfloat16
    x = np.ascontiguousarray(np.asarray(x, dtype=np.float32))
    w_attn = np.asarray(w_attn, dtype=np.float32)
    b_attn = np.asarray(b_attn, dtype=np.float32)
    w_proj = np.asarray(w_proj, dtype=np.float32)

    # triangular causal mask tile [128, 128]: p > f -> NEG
    pp, ff = np.meshgrid(np.arange(P), np.arange(P), indexing="ij")
    mask = np.where(pp > ff, np.float32(NEG), np.float32(0.0)).astype(np.float32)

    in_maps = []
    for core in range(N_CORES):
        b, hg = divmod(core, 4)
        hs = 3 * hg  # first head of this core
        # column bases in the 2304-wide qkv dim
        q0, k0, v0 = 64 * hs, CDIM + 64 * hs, 2 * CDIM + 64 * hs
        # M-blocks: b0=[Q0|Q1] b1=[K0|K1] b2=[Q2|K2], then V (192)
        w = np.concatenate(
            [
                w_attn[:, q0 : q0 + 128],
                w_attn[:, k0 : k0 + 128],
                w_attn[:, q0 + 128 : q0 + 192],
                w_attn[:, k0 + 128 : k0 + 192],
                w_attn[:, v0 : v0 + 192],
            ],
            axis=1,
        )
        bqk = np.zeros((3, P), dtype=np.float32)
        bqk[0] = b_attn[q0 : q0 + 128]
        bqk[1] = b_attn[k0 : k0 + 128]
        bqk[2, 0:64] = b_attn[q0 + 128 : q0 + 192]
        bqk[2, 64:128] = b_attn[k0 + 128 : k0 + 192]
        bvb = np.tile(b_attn[v0 : v0 + 192][None, :], (P, 1)).astype(np.float32)
        wp = np.ascontiguousarray(w_proj[64 * hs : 64 * hs + 192, :])
        in_maps.append(
            {
                "x": np.ascontiguousarray(x[b].T.astype(npm)),
                "w": np.ascontiguousarray(w.astype(npm)),
                "bqk": bqk,
                "bvb": bvb,
                "wp": wp.astype(npm),
                "mask": mask,
                "ones_r": np.ones((1, 64), dtype=np.float32),
            }
        )
    return in_maps


def run(x, w_attn, b_attn, w_proj, b_proj, trace=False):
    nc = _get_nc()
    in_maps = _host_inputs(x, w_attn, b_attn, w_proj)
    res = bass_utils.run_bass_kernel_spmd(
        nc, in_maps, core_ids=list(range(N_CORES)), trace=trace
    )
    B = 2
    outT = np.zeros((B, CDIM, T), dtype=np.float64)
    for core in range(N_CORES):
        outT[core // 4] += res.results[core]["out"].astype(np.float64)
    out = outT.transpose(0, 2, 1)
    out = out + np.asarray(b_proj, dtype=np.float64)[None, None, :]
    return np.ascontiguousarray(out.astype(np.float32)), res


def kernel(x, w_attn, b_attn, w_proj, b_proj):
    out, _ = run(x, w_attn, b_attn, w_proj, b_proj, trace=False)
    return out
